# revision 1
# baseline (speedup 1.0000x reference)
"""Trainium2 Bass kernel for nn_ComparisonLayer.

Computes, for x:(L,B,D) with L=512,B=2,D=256,C=128,O=64:
    xb  = layernorm(transpose(x,(1,0,2)))          # (B,L,D)
    a   = xb@w1+b1 ; b = xb@w2+b2                  # (B,L,C)
    out[b,l,m,o] = sum_c a[b,l,c]*b[b,m,c]*w3[c,o] + b3[o]
                   + (a@w4)[b,l,o] - (b@w4)[b,m,o] # (B,L,L,O)

Sharding: 8 cores, core k handles batch k//4 and l-block (k%4)*128.
Each core writes out[b, lblk:lblk+128, :, :] = (128, 512*64) with l on
partitions and (m,o) contiguous on the free dim -> contiguous HBM writes.

Per chunk (8 m's = 512 free columns) three matmuls accumulate in PSUM:
  1. main : lhsT=aT (C=128,l=128), rhs=R_chunk[c,(m,o)] = bT[c,m]*w3[c,o]
  2. a4   : lhsT=a4T (O=64,l=128), rhs=I64 tiled (constant)  -> +a4[l,o]
  3. row  : lhsT=ones (1,128),     rhs=(b3-b4) flat slice    -> +b3[o]-b4[m,o]
R_chunk is built on the vector engine with stride-0 broadcast APs.
"""

import os
import numpy as np
import ml_dtypes

import concourse.bacc as bacc
import concourse.bass as bass
import concourse.mybir as mybir
import concourse.tile as tile
from concourse.bass_utils import run_bass_kernel_spmd

L, B, D, C, O = 512, 2, 256, 128, 64
NCORES = 8
LBLK = 128                   # l rows per core
MOCT = 8                     # m values per chunk
CHW = MOCT * O               # 512 = chunk width (free columns)
NCH = L // MOCT              # 64 chunks
LN_EPS = 1e-5

F32 = mybir.dt.float32
BF16 = mybir.dt.bfloat16
F32R = mybir.dt.float32r

# "f32r" (default): matmul operands float32r (full PE rate at N>=512,
# near-fp32 accuracy). "bf16": operands bf16.
MM_MODE = os.environ.get("BASS_MM_MODE", "f32r")


def _build(mode: str):
    # storage dtype of main-matmul operands; producers round on write
    cdt = F32R if mode == "f32r" else BF16
    npdt = np.float32 if mode == "f32r" else ml_dtypes.bfloat16

    nc = bacc.Bacc("TRN2", target_bir_lowering=False, debug=False)

    xb_d = nc.dram_tensor("xb", (L, D), F32, kind="ExternalInput")
    xa_d = nc.dram_tensor("xa", (LBLK, D), F32, kind="ExternalInput")
    w1_d = nc.dram_tensor("w1g", (D, C), F32, kind="ExternalInput")
    w2_d = nc.dram_tensor("w2g", (D, C), F32, kind="ExternalInput")
    b1_d = nc.dram_tensor("b1e", (C, 1), F32, kind="ExternalInput")
    b2_d = nc.dram_tensor("b2e", (C, 1), F32, kind="ExternalInput")
    w3_d = nc.dram_tensor("w3c", (C, O), F32, kind="ExternalInput")
    w4_d = nc.dram_tensor("w4f", (C, O), F32, kind="ExternalInput")
    b3_d = nc.dram_tensor("b3f", (1, O), F32, kind="ExternalInput")
    id128_d = nc.dram_tensor("id128", (128, 128), F32, kind="ExternalInput")
    i64r_d = nc.dram_tensor("i64rep", (O, CHW), cdt, kind="ExternalInput")
    out_d = nc.dram_tensor("out", (LBLK, L * O), F32, kind="ExternalOutput")

    NT = L // 128  # xb tiles

    with tile.TileContext(nc) as tc:
        with (
            tc.tile_pool(name="const", bufs=1) as cp,
            tc.tile_pool(name="work", bufs=2) as wp,
            tc.tile_pool(name="rpool", bufs=6) as rp,
            tc.tile_pool(name="opool", bufs=6) as op,
            tc.tile_pool(name="ps_pre", bufs=4, space="PSUM") as pp,
            tc.tile_pool(name="ps_main", bufs=4, space="PSUM") as pm,
        ):
            # ---------- loads ----------
            xsb = cp.tile([128, NT, D], F32)
            nc.sync.dma_start(xsb[:], xb_d.rearrange("(t p) d -> p t d", p=128))
            xasb = cp.tile([128, D], F32)
            nc.sync.dma_start(xasb[:], xa_d[:])
            w1s = cp.tile([128, 2, C], F32)
            nc.sync.dma_start(w1s[:], w1_d.rearrange("(h p) c -> p h c", p=128))
            w2s = cp.tile([128, 2, C], F32)
            nc.sync.dma_start(w2s[:], w2_d.rearrange("(h p) c -> p h c", p=128))
            b1s = cp.tile([C, 1], F32)
            nc.sync.dma_start(b1s[:], b1_d[:])
            b2s = cp.tile([C, 1], F32)
            nc.sync.dma_start(b2s[:], b2_d[:])
            w3s = cp.tile([C, O], F32)
            nc.sync.dma_start(w3s[:], w3_d[:])
            w4s = cp.tile([C, O], F32)
            nc.sync.dma_start(w4s[:], w4_d[:])
            b3s = cp.tile([1, O], F32)
            nc.sync.dma_start(b3s[:], b3_d[:])
            id128 = cp.tile([128, 128], F32)
            nc.sync.dma_start(id128[:], id128_d[:])
            i64r = cp.tile([O, CHW], cdt)
            nc.sync.dma_start(i64r[:], i64r_d[:])

            epsp = cp.tile([128, 1], F32)
            nc.vector.memset(epsp[:], LN_EPS)
            zerop = cp.tile([128, 1], F32)
            nc.vector.memset(zerop[:], 0.0)
            ones_f = cp.tile([1, 128], F32)
            nc.vector.memset(ones_f[:], 1.0)
            ones_c = cp.tile([1, 128], cdt)
            nc.vector.tensor_copy(ones_c[:], ones_f[:])

            # ---------- layernorm ----------
            def layer_norm(dst, src):
                # dst, src: (128, D) f32
                s = wp.tile([128, 1], F32, tag="ln_s")
                nc.vector.tensor_reduce(
                    s[:], src, axis=mybir.AxisListType.X, op=mybir.AluOpType.add
                )
                nmu = wp.tile([128, 1], F32, tag="ln_nmu")
                nc.scalar.mul(nmu[:], s[:], -1.0 / D)
                cen = wp.tile([128, D], F32, tag="ln_cen")
                nc.scalar.activation(
                    cen[:], src, mybir.ActivationFunctionType.Identity,
                    bias=nmu[:],
                )
                sq = wp.tile([128, D], F32, tag="ln_sq")
                vs = wp.tile([128, 1], F32, tag="ln_vs")
                nc.scalar.activation(
                    sq[:], cen[:], mybir.ActivationFunctionType.Square,
                    bias=zerop[:], accum_out=vs[:],
                )
                std = wp.tile([128, 1], F32, tag="ln_std")
                nc.scalar.activation(
                    std[:], vs[:], mybir.ActivationFunctionType.Sqrt,
                    bias=epsp[:], scale=1.0 / D,
                )
                rstd = wp.tile([128, 1], F32, tag="ln_rstd")
                nc.vector.reciprocal(rstd[:], std[:])
                nc.scalar.activation(
                    dst, cen[:], mybir.ActivationFunctionType.Copy,
                    scale=rstd[:],
                )

            xn = cp.tile([128, NT, D], F32)
            for t in range(NT):
                layer_norm(xn[:, t, :], xsb[:, t, :])
            xna = cp.tile([128, D], F32)
            layer_norm(xna[:], xasb[:])

            # ---------- transposes: xnT (d on partitions) ----------
            xnT = cp.tile([128, 2, L], F32)     # [d_in_half, h, m]
            for t in range(NT):
                for h in range(2):
                    tp = pp.tile([128, 128], F32, tag="pre")
                    nc.tensor.transpose(
                        tp[:], xn[:, t, h * 128:(h + 1) * 128], id128[:]
                    )
                    nc.scalar.copy(xnT[:, h, t * 128:(t + 1) * 128], tp[:])
            xnaT = cp.tile([128, 2, 128], F32)
            for h in range(2):
                tp = pp.tile([128, 128], F32, tag="pre")
                nc.tensor.transpose(
                    tp[:], xna[:, h * 128:(h + 1) * 128], id128[:]
                )
                nc.scalar.copy(xnaT[:, h, :], tp[:])

            # ---------- bT (C, L) and aT (C, 128), fp32 ----------
            bps = pp.tile([C, L], F32, tag="pre")
            for h in range(2):
                nc.tensor.matmul(
                    bps[:], w2s[:, h, :], xnT[:, h, :],
                    start=(h == 0), stop=(h == 1),
                )
            bT = cp.tile([C, L], F32)
            nc.vector.tensor_scalar_add(bT[:], bps[:], b2s[:])

            aps_ = pp.tile([C, 128], F32, tag="pre")
            for h in range(2):
                nc.tensor.matmul(
                    aps_[:], w1s[:, h, :], xnaT[:, h, :],
                    start=(h == 0), stop=(h == 1),
                )
            aT = cp.tile([C, 128], F32)
            nc.vector.tensor_scalar_add(aT[:], aps_[:], b1s[:])

            # main-matmul lhsT in mm dtype (rounded on write)
            aT_c = cp.tile([C, 128], cdt)
            nc.vector.tensor_copy(aT_c[:], aT[:])
            bT_c = bT   # only ever a DVE tensor_tensor input

            # ---------- a4 (l,o) -> a4T (o,l) ----------
            a4ps = pp.tile([128, O], F32, tag="pre")
            nc.tensor.matmul(a4ps[:], aT[:], w4s[:], start=True, stop=True)
            a4sb = cp.tile([128, O], F32)
            nc.scalar.copy(a4sb[:], a4ps[:])
            a4Tps = pp.tile([O, 128], F32, tag="pre")
            nc.tensor.transpose(a4Tps[:], a4sb[:], id128[:])
            a4T = cp.tile([O, 128], cdt)
            nc.scalar.copy(a4T[:], a4Tps[:])

            # ---------- row64 = (b3 - b4) flattened (1, L*O) ----------
            b3cps = pp.tile([128, O], F32, tag="pre")
            nc.tensor.matmul(b3cps[:], ones_f[:], b3s[:], start=True, stop=True)
            b3c = cp.tile([128, O], F32)
            nc.scalar.copy(b3c[:], b3cps[:])

            negb4 = cp.tile([128, NT, O], cdt)
            for mt in range(NT):
                b4ps = pp.tile([128, O], F32, tag="pre")
                nc.tensor.matmul(
                    b4ps[:], bT[:, mt * 128:(mt + 1) * 128], w4s[:],
                    start=True, stop=True,
                )
                nc.vector.tensor_sub(negb4[:, mt, :], b3c[:], b4ps[:])

            row64 = cp.tile([1, L * O], cdt)
            for mt in range(NT):
                dst = row64[0:1, mt * 128 * O:(mt + 1) * 128 * O]
                nc.gpsimd.dma_start(dst, negb4[:, mt, :])

            # ---------- main loop over chunks ----------
            for ch in range(NCH):
                rch = rp.tile([C, MOCT, O], cdt, tag="rch")
                in0 = bT_c[:, ch * MOCT:(ch + 1) * MOCT].unsqueeze(2) \
                    .broadcast_to((C, MOCT, O))
                in1 = w3s[:].unsqueeze(1).broadcast_to((C, MOCT, O))
                nc.vector.tensor_mul(rch[:], in0, in1)

                ps = pm.tile([128, CHW], F32, tag="ps")
                nc.tensor.matmul(ps[:], aT_c[:], rch[:],
                                 start=True, stop=False)
                nc.tensor.matmul(ps[:], a4T[:], i64r[:],
                                 start=False, stop=False)
                nc.tensor.matmul(
                    ps[:], ones_c[:],
                    row64[0:1, ch * CHW:(ch + 1) * CHW],
                    start=False, stop=True,
                )

                ob = op.tile([128, CHW], F32, tag="ob")
                nc.scalar.copy(ob[:], ps[:])
                nc.sync.dma_start(out_d[:, ch * CHW:(ch + 1) * CHW], ob[:])

    nc.compile()
    return nc, npdt


_CACHE = {}


def _get_nc(mode):
    if mode not in _CACHE:
        _CACHE[mode] = _build(mode)
    return _CACHE[mode]


def _make_in_maps(x, ln_gamma, ln_beta, w1, b1, w2, b2, w3, b3, w4, npdt):
    x = np.ascontiguousarray(x, dtype=np.float32)
    g = np.asarray(ln_gamma, np.float32)
    be = np.asarray(ln_beta, np.float32)
    w1 = np.asarray(w1, np.float32)
    w2 = np.asarray(w2, np.float32)
    # fold the LN affine into the first-layer weights:
    # (xn*g + be) @ w = xn @ (g[:,None]*w) + be @ w
    w1g = g[:, None] * w1
    w2g = g[:, None] * w2
    b1e = (np.asarray(b1, np.float32) + be @ w1).reshape(C, 1)
    b2e = (np.asarray(b2, np.float32) + be @ w2).reshape(C, 1)
    w3c = np.ascontiguousarray(np.asarray(w3, np.float32))
    w4f = np.ascontiguousarray(np.asarray(w4, np.float32))
    b3f = np.asarray(b3, np.float32).reshape(1, O)
    id128 = np.eye(128, dtype=np.float32)
    i64rep = np.ascontiguousarray(
        np.tile(np.eye(O, dtype=np.float32), (1, MOCT)).astype(npdt))

    in_maps = []
    for k in range(NCORES):
        bi, q = k // (NCORES // B), k % (NCORES // B)
        in_maps.append({
            "xb": np.ascontiguousarray(x[:, bi, :]),
            "xa": np.ascontiguousarray(x[q * LBLK:(q + 1) * LBLK, bi, :]),
            "w1g": w1g, "w2g": w2g, "b1e": b1e, "b2e": b2e,
            "w3c": w3c, "w4f": w4f, "b3f": b3f,
            "id128": id128, "i64rep": i64rep,
        })
    return in_maps


def kernel_run(inputs, trace=False, mode=None):
    mode = mode or MM_MODE
    nc, npdt = _get_nc(mode)
    in_maps = _make_in_maps(npdt=npdt, **inputs)
    res = run_bass_kernel_spmd(
        nc, in_maps, core_ids=list(range(NCORES)), trace=trace,
    )
    out = np.empty((B, L, L, O), dtype=np.float32)
    for k in range(NCORES):
        bi, q = k // (NCORES // B), k % (NCORES // B)
        out[bi, q * LBLK:(q + 1) * LBLK] = \
            res.results[k]["out"].reshape(LBLK, L, O)
    return out, res


def kernel(**inputs) -> np.ndarray:
    out, _ = kernel_run(inputs, trace=False)
    return out



# revision 5
# speedup vs baseline: 1.0231x; 1.0231x over previous
"""Trainium2 Bass kernel for nn_ComparisonLayer (per-o restructure).

Computes, for x:(L,B,D) with L=512,B=2,D=256,C=128,O=64:
    xb  = layernorm(transpose(x,(1,0,2)))          # (B,L,D)
    a   = xb@w1+b1 ; b = xb@w2+b2                  # (B,L,C)
    out[b,l,m,o] = sum_c a[b,l,c]*b[b,m,c]*w3[c,o] + b3[o]
                   + (a@w4)[b,l,o] - (b@w4)[b,m,o] # (B,L,L,O)

Sharding: 8 cores, core k handles batch k//4 and l-block (k%4)*128.

Per-o formulation (o = output channel, 64 iterations):
    out[l, o, m] = (aT * w3[:,o]).T @ bT          # one K=128 N=512 matmul
                 + a4[l, o]                        # per-partition bias in drain
                 + (b3[o] - b4T[o, m])             # rank-1 ones-matmul, 4x
                                                   # row-tiled (concurrent)
Device output layout is (l, (o, m)) in bf16; host transposes to (l, m, o)
and upcasts to fp32.  Engines: PE does 64 main matmuls + 16 groups of 4
concurrent rank-1 matmuls; aw-builds and PSUM drains rotate across
Scalar/Vector/GpSimd; output leaves via 8 x 1MiB HWDGE DMAs.
"""

import numpy as np
import ml_dtypes

import concourse.bacc as bacc
import concourse.bass as bass
import concourse.mybir as mybir
import concourse.tile as tile
from concourse.bass_utils import run_bass_kernel_spmd

L, B, D, C, O = 512, 2, 256, 128, 64
NCORES = 8
LBLK = 128                   # l rows per core
NT = 4                       # full-L tiles of 128 rows
NTL = 5                      # + one xa tile
OBLK = 8                     # o's per output DMA block
NBLK = O // OBLK             # 8 blocks
LN_EPS = 1e-5

F32 = mybir.dt.float32
BF16 = mybir.dt.bfloat16

# packed bf16 weights layout (columns)
WBF_W1 = 0          # [0:256)   w1g halves (h p) c -> p (h c)
WBF_W2 = 256        # [256:512) w2g halves
WBF_W4 = 512        # [512:576) w4 (C, O)
WBF_ID = 576        # [576:704) id128
WBF_N = 704
# packed f32 weights layout
WF_W3 = 0           # [0:64)  w3 (C, O)
WF_B1 = 64          # b1e
WF_B2 = 65          # b2e
WF_B3 = 66          # b3 on partitions 0..63
WF_N = 67


def _build():
    nc = bacc.Bacc("TRN2", target_bir_lowering=False, debug=False)

    xall_d = nc.dram_tensor("xall", (128, NTL, D), F32, kind="ExternalInput")
    wbf_d = nc.dram_tensor("wbf", (128, WBF_N), BF16, kind="ExternalInput")
    wf32_d = nc.dram_tensor("wf32", (128, WF_N), F32, kind="ExternalInput")
    out_d = nc.dram_tensor("out", (LBLK, O * L), BF16, kind="ExternalOutput")

    AX = mybir.AxisListType.X
    ALU = mybir.AluOpType
    ACT = mybir.ActivationFunctionType

    with tile.TileContext(nc) as tc:
        with (
            tc.tile_pool(name="const", bufs=1) as cp,
            tc.tile_pool(name="work", bufs=2) as wp,
            tc.tile_pool(name="aw", bufs=4) as awp,
            tc.tile_pool(name="ob", bufs=2) as obp,
            tc.tile_pool(name="ps_pre", bufs=3, space="PSUM") as pp,
            tc.tile_pool(name="ps_main", bufs=5, space="PSUM") as pm,
        ):
            # ---------- loads ----------
            xall = cp.tile([128, NTL, D], F32)
            nc.sync.dma_start(xall[:], xall_d[:])
            wbf = cp.tile([128, WBF_N], BF16)
            nc.sync.dma_start(wbf[:], wbf_d[:])
            wf32 = cp.tile([128, WF_N], F32)
            nc.sync.dma_start(wf32[:], wf32_d[:])

            id128 = wbf[:, WBF_ID:WBF_ID + 128]
            w4s = wbf[:, WBF_W4:WBF_W4 + O]

            epsp = cp.tile([128, 1], F32)
            nc.vector.memset(epsp[:], LN_EPS)
            onesb = cp.tile([128, 128], BF16)
            nc.vector.memset(onesb[:], 1.0)

            # ---------- layernorm (batched over the 5 tiles) ----------
            s5 = wp.tile([128, NTL], F32, tag="s5")
            vs5 = wp.tile([128, NTL], F32, tag="vs5")
            for t in range(NTL):
                nc.vector.tensor_reduce(
                    s5[:, t:t + 1], xall[:, t, :], axis=AX, op=ALU.add
                )
                sq = wp.tile([128, D], F32, tag="sq")
                nc.scalar.activation(
                    sq[:], xall[:, t, :], ACT.Square,
                    accum_out=vs5[:, t:t + 1],
                )
            mu5 = wp.tile([128, NTL], F32, tag="mu5")
            nc.vector.tensor_scalar_mul(mu5[:], s5[:], 1.0 / D)
            musq5 = wp.tile([128, NTL], F32, tag="musq5")
            nc.vector.tensor_tensor(musq5[:], mu5[:], mu5[:], op=ALU.mult)
            var5 = wp.tile([128, NTL], F32, tag="var5")
            nc.vector.scalar_tensor_tensor(
                var5[:], vs5[:], 1.0 / D, musq5[:],
                op0=ALU.mult, op1=ALU.subtract,
            )
            std5 = wp.tile([128, NTL], F32, tag="std5")
            nc.scalar.activation(std5[:], var5[:], ACT.Sqrt, bias=epsp[:])
            rstd5 = wp.tile([128, NTL], F32, tag="rstd5")
            nc.vector.reciprocal(rstd5[:], std5[:])
            mrs5 = wp.tile([128, NTL], F32, tag="mrs5")
            nc.vector.tensor_tensor(mrs5[:], mu5[:], rstd5[:], op=ALU.mult)

            t1 = wp.tile([128, NTL, D], F32, tag="t1")
            nc.vector.tensor_tensor(
                t1[:], xall[:],
                rstd5[:].unsqueeze(2).broadcast_to((128, NTL, D)),
                op=ALU.mult,
            )
            xn = cp.tile([128, NTL, D], BF16)
            nc.vector.tensor_tensor(
                xn[:], t1[:],
                mrs5[:].unsqueeze(2).broadcast_to((128, NTL, D)),
                op=ALU.subtract,
            )

            # ---------- transposes: xnT[dh, h, t, lj] ----------
            xnT = cp.tile([128, 2, NTL, 128], BF16)
            for t in range(NTL):
                for h in range(2):
                    tp = pp.tile([128, 128], BF16, tag="pre")
                    nc.tensor.transpose(
                        tp[:], xn[:, t, h * 128:(h + 1) * 128], id128
                    )
                    nc.scalar.copy(xnT[:, h, t, :], tp[:])

            # ---------- bT (C, L) and aT (C, 128) ----------
            bps = pp.tile([C, L], F32, tag="pre")
            for h in range(2):
                nc.tensor.matmul(
                    bps[:], wbf[:, WBF_W2 + h * 128:WBF_W2 + (h + 1) * 128],
                    xnT[:, h, 0:NT, :], start=(h == 0), stop=(h == 1),
                )
            bT_c = cp.tile([C, L], BF16)
            nc.vector.tensor_scalar_add(bT_c[:], bps[:], wf32[:, WF_B2:WF_B2 + 1])

            aps = pp.tile([C, 128], F32, tag="pre")
            for h in range(2):
                nc.tensor.matmul(
                    aps[:], wbf[:, WBF_W1 + h * 128:WBF_W1 + (h + 1) * 128],
                    xnT[:, h, NT, :], start=(h == 0), stop=(h == 1),
                )
            aT_sb = cp.tile([C, 128], F32)
            nc.vector.tensor_scalar_add(aT_sb[:], aps[:], wf32[:, WF_B1:WF_B1 + 1])
            aT_c = cp.tile([C, 128], BF16)
            nc.vector.tensor_copy(aT_c[:], aT_sb[:])

            # ---------- a4 (l, o) fp32: drain bias ----------
            a4ps = pp.tile([128, O], F32, tag="pre")
            nc.tensor.matmul(a4ps[:], aT_c[:], w4s, start=True, stop=True)
            a4sb = cp.tile([128, O], F32)
            nc.scalar.copy(a4sb[:], a4ps[:])

            # ---------- rowRep: (b3 - b4T)[o, m] replicated on partitions
            # 0/32/64/96 for the 4x row-tiled rank-1 matmuls ----------
            b4ps = pp.tile([O, L], F32, tag="pre")
            nc.tensor.matmul(b4ps[:], w4s, bT_c[:], start=True, stop=True)
            negb4T = cp.tile([O, L], BF16)
            nc.vector.tensor_scalar(
                negb4T[:], b4ps[:], -1.0, wf32[0:O, WF_B3:WF_B3 + 1],
                op0=ALU.mult, op1=ALU.add,
            )
            rowRep = cp.tile([128, O * L], BF16)
            for r in range(4):
                nc.gpsimd.dma_start(
                    rowRep[32 * r:32 * r + 1, :], negb4T[:, :]
                )

            # ---------- main loop: 16 groups of 4 o's ----------
            # gpsimd cannot touch PSUM: it builds aw (SBUF->SBUF); PSUM
            # drains alternate between scalar and vector.
            def build_aw(o, dst):
                w3col = wf32[:, WF_W3 + o:WF_W3 + o + 1]
                nc.gpsimd.tensor_scalar_mul(dst, aT_sb[:], w3col)

            def drain(o, dst, ps):
                a4col = a4sb[:, o:o + 1]
                if o % 2 == 0:
                    nc.scalar.add(dst, ps, a4col)
                else:
                    nc.vector.tensor_scalar_add(dst, ps, a4col)

            ob = None
            for g in range(O // 4):
                pss = []
                for j in range(4):
                    o = 4 * g + j
                    aw = awp.tile([C, 128], BF16, tag="aw")
                    build_aw(o, aw[:])
                    ps = pm.tile([128, L], F32, tag="ps")
                    nc.tensor.matmul(ps[:], aw[:], bT_c[:],
                                     start=True, stop=False)
                    pss.append(ps)
                for j in range(4):
                    o = 4 * g + j
                    nc.tensor.matmul(
                        pss[j][:], onesb[32 * j:32 * j + 1, :],
                        rowRep[32 * j:32 * j + 1, o * L:(o + 1) * L],
                        start=False, stop=True, tile_position=(32 * j, 0),
                    )
                for j in range(4):
                    o = 4 * g + j
                    if o % OBLK == 0:
                        ob = obp.tile([128, OBLK * L], BF16, tag="ob")
                    sl = ob[:, (o % OBLK) * L:(o % OBLK + 1) * L]
                    drain(o, sl, pss[j][:])
                    if o % OBLK == OBLK - 1:
                        blk = o // OBLK
                        nc.sync.dma_start(
                            out_d[:, blk * OBLK * L:(blk + 1) * OBLK * L],
                            ob[:],
                        )

    nc.compile()
    return nc


_CACHE = {}


def _get_nc():
    if "nc" not in _CACHE:
        _CACHE["nc"] = _build()
    return _CACHE["nc"]


def _make_in_maps(x, ln_gamma, ln_beta, w1, b1, w2, b2, w3, b3, w4):
    x = np.ascontiguousarray(x, dtype=np.float32)
    g = np.asarray(ln_gamma, np.float32)
    be = np.asarray(ln_beta, np.float32)
    w1 = np.asarray(w1, np.float32)
    w2 = np.asarray(w2, np.float32)
    # fold the LN affine into the first-layer weights:
    # (xn*g + be) @ w = xn @ (g[:,None]*w) + be @ w
    w1g = g[:, None] * w1
    w2g = g[:, None] * w2
    b1e = (np.asarray(b1, np.float32) + be @ w1).reshape(C, 1)
    b2e = (np.asarray(b2, np.float32) + be @ w2).reshape(C, 1)
    w3c = np.asarray(w3, np.float32)
    w4f = np.asarray(w4, np.float32)
    b3f = np.asarray(b3, np.float32)

    bf = ml_dtypes.bfloat16
    wbf = np.zeros((128, WBF_N), dtype=bf)
    wbf[:, WBF_W1:WBF_W1 + 256] = \
        w1g.reshape(2, 128, C).transpose(1, 0, 2).reshape(128, 256).astype(bf)
    wbf[:, WBF_W2:WBF_W2 + 256] = \
        w2g.reshape(2, 128, C).transpose(1, 0, 2).reshape(128, 256).astype(bf)
    wbf[:, WBF_W4:WBF_W4 + O] = w4f.astype(bf)
    wbf[:, WBF_ID:WBF_ID + 128] = np.eye(128, dtype=np.float32).astype(bf)

    wf32 = np.zeros((128, WF_N), dtype=np.float32)
    wf32[:, WF_W3:WF_W3 + O] = w3c
    wf32[:, WF_B1] = b1e[:, 0]
    wf32[:, WF_B2] = b2e[:, 0]
    wf32[0:O, WF_B3] = b3f

    in_maps = []
    for k in range(NCORES):
        bi, q = k // (NCORES // B), k % (NCORES // B)
        xb = x[:, bi, :]                                   # (L, D)
        xtiles = xb.reshape(NT, 128, D).transpose(1, 0, 2)  # (128, NT, D)
        xa = xb[q * LBLK:(q + 1) * LBLK, :][:, None, :]     # (128, 1, D)
        xall = np.ascontiguousarray(
            np.concatenate([xtiles, xa], axis=1))           # (128, NTL, D)
        in_maps.append({"xall": xall, "wbf": wbf, "wf32": wf32})
    return in_maps


def kernel_run(inputs, trace=False):
    nc = _get_nc()
    in_maps = _make_in_maps(**inputs)
    res = run_bass_kernel_spmd(
        nc, in_maps, core_ids=list(range(NCORES)), trace=trace,
    )
    out = np.empty((B, L, L, O), dtype=np.float32)
    for k in range(NCORES):
        bi, q = k // (NCORES // B), k % (NCORES // B)
        blk = np.asarray(res.results[k]["out"]).astype(np.float32)
        out[bi, q * LBLK:(q + 1) * LBLK] = \
            blk.reshape(LBLK, O, L).transpose(0, 2, 1)
    return out, res


def kernel(**inputs) -> np.ndarray:
    out, _ = kernel_run(inputs, trace=False)
    return out


# revision 7
# speedup vs baseline: 1.7137x; 1.6750x over previous
"""Trainium2 Bass kernel for nn_ComparisonLayer (per-o restructure).

Computes, for x:(L,B,D) with L=512,B=2,D=256,C=128,O=64:
    xb  = layernorm(transpose(x,(1,0,2)))          # (B,L,D)
    a   = xb@w1+b1 ; b = xb@w2+b2                  # (B,L,C)
    out[b,l,m,o] = sum_c a[b,l,c]*b[b,m,c]*w3[c,o] + b3[o]
                   + (a@w4)[b,l,o] - (b@w4)[b,m,o] # (B,L,L,O)

Sharding: 8 cores, core k handles batch k//4 and l-block (k%4)*128.

Per-o formulation (o = output channel, 64 iterations):
    out[l, o, m] = (aT * w3[:,o]).T @ bT          # one K=128 N=512 matmul
                 + a4[l, o]                        # per-partition bias in drain
                 + (b3[o] - b4T[o, m])             # rank-1 ones-matmul, 4x
                                                   # row-tiled (concurrent)
Device output layout is (l, (o, m)) in bf16; host transposes to (l, m, o)
and upcasts to fp32.  Engines: PE does 64 main matmuls + 16 groups of 4
concurrent rank-1 matmuls; aw-builds and PSUM drains rotate across
Scalar/Vector/GpSimd; output leaves via 8 x 1MiB HWDGE DMAs.
"""

import numpy as np
import ml_dtypes

import concourse.bacc as bacc
import concourse.bass as bass
import concourse.mybir as mybir
import concourse.tile as tile
from concourse.bass_utils import run_bass_kernel_spmd

L, B, D, C, O = 512, 2, 256, 128, 64
NCORES = 8
LBLK = 128                   # l rows per core
NT = 4                       # full-L tiles of 128 rows
NTL = 5                      # + one xa tile
OBLK = 8                     # o's per output DMA block
NBLK = O // OBLK             # 8 blocks
LN_EPS = 1e-5

F32 = mybir.dt.float32
BF16 = mybir.dt.bfloat16

# packed bf16 weights layout (columns)
WBF_W1 = 0          # [0:256)   w1g halves (h p) c -> p (h c)
WBF_W2 = 256        # [256:512) w2g halves
WBF_W4 = 512        # [512:576) w4 (C, O)
WBF_ID = 576        # [576:704) id128
WBF_N = 704
# packed f32 weights layout
WF_W3 = 0           # [0:64)  w3 (C, O)
WF_B1 = 64          # b1e
WF_B2 = 65          # b2e
WF_B3 = 66          # b3 on partitions 0..63
WF_N = 67


def _build():
    nc = bacc.Bacc("TRN2", target_bir_lowering=False, debug=False)

    xall_d = nc.dram_tensor("xall", (128, NTL, D), F32, kind="ExternalInput")
    wbf_d = nc.dram_tensor("wbf", (128, WBF_N), BF16, kind="ExternalInput")
    wf32_d = nc.dram_tensor("wf32", (128, WF_N), F32, kind="ExternalInput")
    out_d = nc.dram_tensor("out", (LBLK, O * L), BF16, kind="ExternalOutput")

    AX = mybir.AxisListType.X
    ALU = mybir.AluOpType
    ACT = mybir.ActivationFunctionType

    with tile.TileContext(nc) as tc:
        with (
            tc.tile_pool(name="const", bufs=1) as cp,
            tc.tile_pool(name="work", bufs=2) as wp,
            tc.tile_pool(name="aw", bufs=2) as awp,
            tc.tile_pool(name="ob", bufs=2) as obp,
            tc.tile_pool(name="ps_pre", bufs=3, space="PSUM") as pp,
            tc.tile_pool(name="ps_main", bufs=5, space="PSUM") as pm,
        ):
            # ---------- loads ----------
            xall = cp.tile([128, NTL, D], F32)
            nc.sync.dma_start(xall[:], xall_d[:])
            wbf = cp.tile([128, WBF_N], BF16)
            nc.sync.dma_start(wbf[:], wbf_d[:])
            wf32 = cp.tile([128, WF_N], F32)
            nc.sync.dma_start(wf32[:], wf32_d[:])

            id128 = wbf[:, WBF_ID:WBF_ID + 128]
            w4s = wbf[:, WBF_W4:WBF_W4 + O]

            epsp = cp.tile([128, 1], F32)
            nc.vector.memset(epsp[:], LN_EPS)
            onesb = cp.tile([128, 128], BF16)
            nc.vector.memset(onesb[:], 1.0)

            # ---------- layernorm (batched over the 5 tiles) ----------
            s5 = wp.tile([128, NTL], F32, tag="s5")
            vs5 = wp.tile([128, NTL], F32, tag="vs5")
            for t in range(NTL):
                nc.vector.tensor_reduce(
                    s5[:, t:t + 1], xall[:, t, :], axis=AX, op=ALU.add
                )
                sq = wp.tile([128, D], F32, tag="sq")
                nc.scalar.activation(
                    sq[:], xall[:, t, :], ACT.Square,
                    accum_out=vs5[:, t:t + 1],
                )
            mu5 = wp.tile([128, NTL], F32, tag="mu5")
            nc.vector.tensor_scalar_mul(mu5[:], s5[:], 1.0 / D)
            musq5 = wp.tile([128, NTL], F32, tag="musq5")
            nc.vector.tensor_tensor(musq5[:], mu5[:], mu5[:], op=ALU.mult)
            var5 = wp.tile([128, NTL], F32, tag="var5")
            nc.vector.scalar_tensor_tensor(
                var5[:], vs5[:], 1.0 / D, musq5[:],
                op0=ALU.mult, op1=ALU.subtract,
            )
            std5 = wp.tile([128, NTL], F32, tag="std5")
            nc.scalar.activation(std5[:], var5[:], ACT.Sqrt, bias=epsp[:])
            rstd5 = wp.tile([128, NTL], F32, tag="rstd5")
            nc.vector.reciprocal(rstd5[:], std5[:])
            mrs5 = wp.tile([128, NTL], F32, tag="mrs5")
            nc.vector.tensor_tensor(mrs5[:], mu5[:], rstd5[:], op=ALU.mult)

            t1 = wp.tile([128, NTL, D], F32, tag="t1")
            nc.vector.tensor_tensor(
                t1[:], xall[:],
                rstd5[:].unsqueeze(2).broadcast_to((128, NTL, D)),
                op=ALU.mult,
            )
            xn = cp.tile([128, NTL, D], BF16)
            nc.vector.tensor_tensor(
                xn[:], t1[:],
                mrs5[:].unsqueeze(2).broadcast_to((128, NTL, D)),
                op=ALU.subtract,
            )

            # ---------- transposes: xnT[dh, h, t, lj] ----------
            xnT = cp.tile([128, 2, NTL, 128], BF16)
            for t in range(NTL):
                for h in range(2):
                    tp = pp.tile([128, 128], BF16, tag="pre")
                    nc.tensor.transpose(
                        tp[:], xn[:, t, h * 128:(h + 1) * 128], id128
                    )
                    nc.scalar.copy(xnT[:, h, t, :], tp[:])

            # ---------- bT (C, L) and aT (C, 128) ----------
            bps = pp.tile([C, L], F32, tag="pre")
            for h in range(2):
                nc.tensor.matmul(
                    bps[:], wbf[:, WBF_W2 + h * 128:WBF_W2 + (h + 1) * 128],
                    xnT[:, h, 0:NT, :], start=(h == 0), stop=(h == 1),
                )
            bT_c = cp.tile([C, L], BF16)
            nc.vector.tensor_scalar_add(bT_c[:], bps[:], wf32[:, WF_B2:WF_B2 + 1])

            aps = pp.tile([C, 128], F32, tag="pre")
            for h in range(2):
                nc.tensor.matmul(
                    aps[:], wbf[:, WBF_W1 + h * 128:WBF_W1 + (h + 1) * 128],
                    xnT[:, h, NT, :], start=(h == 0), stop=(h == 1),
                )
            aT_sb = cp.tile([C, 128], F32)
            nc.vector.tensor_scalar_add(aT_sb[:], aps[:], wf32[:, WF_B1:WF_B1 + 1])
            aT_c = cp.tile([C, 128], BF16)
            nc.vector.tensor_copy(aT_c[:], aT_sb[:])

            # ---------- a4 (l, o) fp32: drain bias ----------
            a4ps = pp.tile([128, O], F32, tag="pre")
            nc.tensor.matmul(a4ps[:], aT_c[:], w4s, start=True, stop=True)
            a4sb = cp.tile([128, O], F32)
            nc.scalar.copy(a4sb[:], a4ps[:])

            # ---------- rowRep: (b3 - b4T)[o, m] replicated on partitions
            # 0/32/64/96 for the 4x row-tiled rank-1 matmuls ----------
            b4ps = pp.tile([O, L], F32, tag="pre")
            nc.tensor.matmul(b4ps[:], w4s, bT_c[:], start=True, stop=True)
            negb4T = cp.tile([O, L], BF16)
            nc.vector.tensor_scalar(
                negb4T[:], b4ps[:], -1.0, wf32[0:O, WF_B3:WF_B3 + 1],
                op0=ALU.mult, op1=ALU.add,
            )
            rowRep = cp.tile([128, O * L], BF16)
            for r in range(4):
                nc.gpsimd.dma_start(
                    rowRep[32 * r:32 * r + 1, :], negb4T[:, :]
                )

            # ---------- main loop: 16 groups of 4 o's ----------
            # gpsimd cannot touch PSUM and its compute ops cost ~2us each;
            # aw is built in batches of 8 o's on the vector engine via a
            # free-dim-broadcast tensor_tensor, and PSUM drains split
            # scalar:vector = 5:3 (scalar ACTIVATE is the cheaper drain).
            def drain(o, dst, ps):
                a4col = a4sb[:, o:o + 1]
                if o % 8 < 5:
                    nc.scalar.add(dst, ps, a4col)
                else:
                    nc.vector.tensor_scalar_add(dst, ps, a4col)

            ob = None
            aw8 = None
            for g in range(O // 4):
                if g % 2 == 0:
                    o0 = 4 * g
                    aw8 = awp.tile([C, OBLK, 128], BF16, tag="aw")
                    nc.vector.tensor_tensor(
                        aw8[:],
                        aT_sb[:].unsqueeze(1).broadcast_to((C, OBLK, 128)),
                        wf32[:, WF_W3 + o0:WF_W3 + o0 + OBLK]
                        .unsqueeze(2).broadcast_to((C, OBLK, 128)),
                        op=ALU.mult,
                    )
                pss = []
                for j in range(4):
                    o = 4 * g + j
                    ps = pm.tile([128, L], F32, tag="ps")
                    nc.tensor.matmul(ps[:], aw8[:, o % OBLK, :], bT_c[:],
                                     start=True, stop=False)
                    pss.append(ps)
                for j in range(4):
                    o = 4 * g + j
                    nc.tensor.matmul(
                        pss[j][:], onesb[32 * j:32 * j + 1, :],
                        rowRep[32 * j:32 * j + 1, o * L:(o + 1) * L],
                        start=False, stop=True, tile_position=(32 * j, 0),
                    )
                for j in range(4):
                    o = 4 * g + j
                    if o % OBLK == 0:
                        ob = obp.tile([128, OBLK * L], BF16, tag="ob")
                    sl = ob[:, (o % OBLK) * L:(o % OBLK + 1) * L]
                    drain(o, sl, pss[j][:])
                    if o % OBLK == OBLK - 1:
                        blk = o // OBLK
                        nc.sync.dma_start(
                            out_d[:, blk * OBLK * L:(blk + 1) * OBLK * L],
                            ob[:],
                        )

    nc.compile()
    return nc


_CACHE = {}


def _get_nc():
    if "nc" not in _CACHE:
        _CACHE["nc"] = _build()
    return _CACHE["nc"]


def _make_in_maps(x, ln_gamma, ln_beta, w1, b1, w2, b2, w3, b3, w4):
    x = np.ascontiguousarray(x, dtype=np.float32)
    g = np.asarray(ln_gamma, np.float32)
    be = np.asarray(ln_beta, np.float32)
    w1 = np.asarray(w1, np.float32)
    w2 = np.asarray(w2, np.float32)
    # fold the LN affine into the first-layer weights:
    # (xn*g + be) @ w = xn @ (g[:,None]*w) + be @ w
    w1g = g[:, None] * w1
    w2g = g[:, None] * w2
    b1e = (np.asarray(b1, np.float32) + be @ w1).reshape(C, 1)
    b2e = (np.asarray(b2, np.float32) + be @ w2).reshape(C, 1)
    w3c = np.asarray(w3, np.float32)
    w4f = np.asarray(w4, np.float32)
    b3f = np.asarray(b3, np.float32)

    bf = ml_dtypes.bfloat16
    wbf = np.zeros((128, WBF_N), dtype=bf)
    wbf[:, WBF_W1:WBF_W1 + 256] = \
        w1g.reshape(2, 128, C).transpose(1, 0, 2).reshape(128, 256).astype(bf)
    wbf[:, WBF_W2:WBF_W2 + 256] = \
        w2g.reshape(2, 128, C).transpose(1, 0, 2).reshape(128, 256).astype(bf)
    wbf[:, WBF_W4:WBF_W4 + O] = w4f.astype(bf)
    wbf[:, WBF_ID:WBF_ID + 128] = np.eye(128, dtype=np.float32).astype(bf)

    wf32 = np.zeros((128, WF_N), dtype=np.float32)
    wf32[:, WF_W3:WF_W3 + O] = w3c
    wf32[:, WF_B1] = b1e[:, 0]
    wf32[:, WF_B2] = b2e[:, 0]
    wf32[0:O, WF_B3] = b3f

    in_maps = []
    for k in range(NCORES):
        bi, q = k // (NCORES // B), k % (NCORES // B)
        xb = x[:, bi, :]                                   # (L, D)
        xtiles = xb.reshape(NT, 128, D).transpose(1, 0, 2)  # (128, NT, D)
        xa = xb[q * LBLK:(q + 1) * LBLK, :][:, None, :]     # (128, 1, D)
        xall = np.ascontiguousarray(
            np.concatenate([xtiles, xa], axis=1))           # (128, NTL, D)
        in_maps.append({"xall": xall, "wbf": wbf, "wf32": wf32})
    return in_maps


def kernel_run(inputs, trace=False):
    nc = _get_nc()
    in_maps = _make_in_maps(**inputs)
    res = run_bass_kernel_spmd(
        nc, in_maps, core_ids=list(range(NCORES)), trace=trace,
    )
    out = np.empty((B, L, L, O), dtype=np.float32)
    for k in range(NCORES):
        bi, q = k // (NCORES // B), k % (NCORES // B)
        blk = np.asarray(res.results[k]["out"]).astype(np.float32)
        out[bi, q * LBLK:(q + 1) * LBLK] = \
            blk.reshape(LBLK, O, L).transpose(0, 2, 1)
    return out, res


def kernel(**inputs) -> np.ndarray:
    out, _ = kernel_run(inputs, trace=False)
    return out
